# revision 30
# baseline (speedup 1.0000x reference)
"""BitConv1d Trainium2 kernel (8 NeuronCores, data-parallel over batch).

Reference semantics (per batch b):
    x_n   = rmsnorm_over_C(x) * gamma
    scale = max(|x_n|) over the WHOLE tensor (global -> AllGather max)
    n     = round(x_n / scale * 127)                          (ints, |n|<=127)
    w_s   = max(mean(|w|), 1e-4)
    w_q   = round(clip(w / w_s, -1, 1))                       (ternary)
    out   = conv1d(n, w_q, pad=3) * (scale/127) * w_s

n is an integer |n|<=127 (exact in bf16) and w_q is ternary (exact in
bf16), so the conv is EXACT integer arithmetic on the PE in bf16 with
fp32 PSUM accumulation.  Rounding uses the fp32 magic-number trick
(+1.5*2^23, RNE) which matches jnp.round.

v3 structure (evidence from perfetto traces of v1/v2):
  * Only rms (1/sqrt(mean x^2 + eps), [128, T] f32, 32KiB/part) stays
    resident in SBUF.  Phase A computes rms + the global abs-max with a
    SINGLE DVE scan per chunk: tensor_tensor_reduce(sq * rms^2,
    accum=max) gives max(x_n^2) while writing only a garbage tile.
    Phase B re-reads x from DRAM (the extra 16MiB hides under 460us of
    matmuls) and computes x*rms on the DVE there.
  * Tile schedules by operand readiness, not emission order, so the
    weight DMAs + phase-B x DMAs carry dependency markers (1-element
    copies from late-phase-A tiles) to keep them off the phase-A DMA
    wire; the weight-quant chain (ACT |w|-accum, ACT magic-round, DVE
    clip/sub) then lands inside the ~45us collective window.
  * The global-max tree uses gpsimd.partition_all_reduce (lib preloaded
    at t=0 via a dummy call); the post-collective scalar broadcast is a
    stride-0 DMA (dram scalar .to_broadcast).
  * Dummy matmuls (dep-chained to the end of phase A) keep the PE HAM
    clock-gate warm through the collective window.
  * Phase B runs chunk-major with 8 PSUM banks double-buffered; the
    scale-free x*rms multiply of the first chunks runs during the
    collective window as soon as their x slices land.
"""

import os
import sys
import types

import numpy as np


def _install_ntff_shim():
    """Make bass_utils' trace path work in containers lacking antenv.axon_hooks."""
    try:
        import antenv.axon_hooks  # noqa: F401
        return
    except ImportError:
        pass
    try:
        from trn_agent_boot.trn_boot import _ntff_profile_via_ctypes

        mod = types.ModuleType("antenv.axon_hooks")
        hook = _ntff_profile_via_ctypes("/opt/axon/libaxon_pjrt.so")
        mod.get_axon_ntff_profile_hook = lambda: hook
        mod.set_axon_ntff_profile_hook = lambda h: None
        sys.modules["antenv.axon_hooks"] = mod
        import antenv

        antenv.axon_hooks = mod
    except Exception:
        pass


_install_ntff_shim()

import concourse.bacc as bacc
import concourse.tile as tile
from concourse import bass_isa, mybir
from concourse.bass_utils import run_bass_kernel_spmd

f32 = mybir.dt.float32
fp16 = mybir.dt.float16
bf16 = mybir.dt.bfloat16

N_CORES = 8
C = 512          # in/out channels
T = 8192         # sequence length
KS = 7           # kernel taps
PAD = 3
NT = 4           # channel tiles of 128
CH = 512         # T-chunk width
NCH = T // CH    # 16
MPAD = 4         # rms margin (any even >= PAD)
TP = T + 2 * MPAD
HB = CH + 2 * PAD  # 518
EPS = 1e-6
QP = 127.0
MAGIC = 12582912.0        # 1.5 * 2**23 : fp32 round-to-nearest-int magic
W_ELEMS = C * C * KS      # 1835008


def _build(apply_gamma: bool):
    Alu = mybir.AluOpType
    ACTF = mybir.ActivationFunctionType
    warm = int(os.environ.get("BITCONV_WARM", "430"))
    use_ag = os.environ.get("BITCONV_AG", "0") == "1"

    nc = bacc.Bacc("TRN2", target_bir_lowering=False, debug=False,
                   num_devices=N_CORES)

    x_ext = nc.dram_tensor("x", [C, T], f32, kind="ExternalInput")
    # host supplies weight transposed to [cin, k, cout] so quantized lhsT
    # tiles are contiguous slices (no on-chip transposes needed)
    w_ext = nc.dram_tensor("w", [C, KS, C], f32, kind="ExternalInput")
    nw_ext = nc.dram_tensor("nw", [C], f32, kind="ExternalInput")
    out_ext = nc.dram_tensor("out", [C, T], f32, kind="ExternalOutput")

    with tile.TileContext(nc) as tc:
        with (
            tc.tile_pool(name="consts", bufs=1) as consts,
            tc.tile_pool(name="rmsres", bufs=1) as rmsres,
            tc.tile_pool(name="wqt", bufs=1) as wqtp,
            tc.tile_pool(name="dram", bufs=1, space="DRAM") as dram,
        ):
            ones_h = consts.tile([128, 256], fp16)
            nc.vector.memset(ones_h[:], 1.0)
            eps_t = consts.tile([128, 1], f32)
            nc.vector.memset(eps_t[:], EPS)
            gamma4 = consts.tile([128, NT], f32)
            if apply_gamma:
                nc.sync.dma_start(
                    out=gamma4[:],
                    in_=nw_ext[:].rearrange("(j p) -> p j", p=128))
            mxbuf = consts.tile([128, NCH], f32)     # max(x_n^2) per chunk
            wsums = consts.tile([128, NT], f32)
            sc128 = consts.tile([128, 1], f32)       # global act scale
            s127 = consts.tile([128, 1], f32)        # 127/scale
            gs4 = consts.tile([128, NT], f32)        # gamma * 127/scale
            ws128 = consts.tile([128, 1], f32)       # weight scale
            winv = consts.tile([128, 1], f32)
            osc = consts.tile([128, 1], f32)         # w_s*scale/127
            scr1 = consts.tile([128, 1], f32)        # gpsimd lib preload dst
            dumr = consts.tile([128, 256], fp16)     # dummy-matmul rhs
            nc.vector.memset(dumr[:], 1.0)

            # resident rms, f32, zero margins so the phase-B halo mul
            # produces exact zeros outside [0, T)
            rms_res = rmsres.tile([128, TP], f32)
            nc.vector.memset(rms_res[:, 0:MPAD], 0.0)
            nc.vector.memset(rms_res[:, MPAD + T:TP], 0.0)

            # ternary weights, bf16, lhsT layout: tile j holds
            # [128 cin, (k, cout)] so slice (k, m) is contiguous
            wqTs = [wqtp.tile([128, KS * C], bf16, name=f"wqT{j}")
                    for j in range(NT)]

            ccin = dram.tile([1, 1], f32)
            if use_ag:
                ccag = dram.tile([N_CORES, 1], f32, addr_space="Shared")
                ccsc = dram.tile([1, 1], f32)
            else:
                ccag = dram.tile([1, 1], f32)

            with (
                tc.tile_pool(name="xin", bufs=2) as xinp,
                tc.tile_pool(name="sq", bufs=2) as sqp,
                tc.tile_pool(name="rms2", bufs=1) as rms2p,
                tc.tile_pool(name="g2", bufs=1) as g2p,
                tc.tile_pool(name="wraw", bufs=4) as wrawp,
                tc.tile_pool(name="wsm", bufs=2) as wsmp,
                tc.tile_pool(name="smal", bufs=4) as smal,
                tc.tile_pool(name="psA", bufs=3, space="PSUM") as psA,
                tc.tile_pool(name="psD", bufs=1, space="PSUM") as psD,
            ):
                # preload the gpsimd custom-op library at t~0 so the
                # partition_all_reduce calls later don't pay the
                # LOAD_LIB + DRAIN (~6us) inside the critical window
                nc.gpsimd.partition_all_reduce(
                    scr1[:], eps_t[:], 128, bass_isa.ReduceOp.max)

                g2 = g2p.tile([128, NT, CH], f32)    # scan garbage dst

                # ---- phase A: rms into resident f32, chunk max(x_n^2) ----
                for ti in range(NCH):
                    t0 = ti * CH
                    xt = xinp.tile([128, NT, CH], f32)
                    nc.sync.dma_start(
                        out=xt[:],
                        in_=x_ext[:, t0:t0 + CH].rearrange(
                            "(j p) t -> p j t", p=128))
                    sq = sqp.tile([128, NT, CH], fp16)
                    nc.scalar.square(sq[:], xt[:])
                    ps = psA.tile([128, CH], f32)
                    for j in range(NT):
                        # accumulate sum_c x^2 on the PE; all-ones lhsT also
                        # broadcasts the result to every partition
                        nc.tensor.matmul(ps[:], ones_h[:, 0:128], sq[:, j, :],
                                         start=(j == 0), stop=(j == NT - 1))
                    rsl = rms_res[:, MPAD + t0:MPAD + t0 + CH]
                    # table rsqrt (max rel err ~4e-5 measured)
                    nc.scalar.activation(out=rsl, in_=ps[:],
                                         func=ACTF.Abs_reciprocal_sqrt,
                                         bias=eps_t[:], scale=1.0 / C)
                    rms2 = rms2p.tile([128, CH], f32)
                    nc.scalar.square(rms2[:], rsl)
                    if apply_gamma:
                        # chunk max must cover gamma^2 * x^2 * rms^2; fold
                        # gamma into phase-B scale, but the max needs it
                        # per-channel -> use gamma'd squares via ttr twice
                        gb2 = smal.tile([128, NT], f32)
                        nc.vector.tensor_mul(gb2[:], gamma4[:], gamma4[:])
                        sqg = sqp.tile([128, NT, CH], fp16)
                        nc.vector.tensor_mul(
                            sqg[:], sq[:],
                            gb2[:, :, None].broadcast_to([128, NT, CH]))
                        nc.vector.tensor_tensor_reduce(
                            out=g2[:], in0=sqg[:],
                            in1=rms2[:, None, :].broadcast_to([128, NT, CH]),
                            scale=1.0, scalar=0.0,
                            op0=Alu.mult, op1=Alu.max,
                            accum_out=mxbuf[:, ti:ti + 1])
                    else:
                        if os.environ.get("BITCONV_TTR", "0") == "1":
                            # ONE DVE scan: garbage out, accum carries
                            # max(x^2 * rms^2) = max(x_n^2) for the chunk
                            nc.vector.tensor_tensor_reduce(
                                out=g2[:], in0=sq[:],
                                in1=rms2[:, None, :].broadcast_to(
                                    [128, NT, CH]),
                                scale=1.0, scalar=0.0,
                                op0=Alu.mult, op1=Alu.max,
                                accum_out=mxbuf[:, ti:ti + 1])
                        else:
                            nc.vector.tensor_mul(
                                g2[:], sq[:],
                                rms2[:, None, :].broadcast_to([128, NT, CH]))
                            # flat view: one AP row, no per-row overhead
                            nc.vector.tensor_reduce(
                                out=mxbuf[:, ti:ti + 1],
                                in_=g2[:].rearrange("p a b -> p (a b)"),
                                axis=mybir.AxisListType.X, op=Alu.max)

                # ---- local max tree + collective (gpsimd only) ----
                # the wire carries the CLAMPED INVERSE scale: global
                # 1/scale = min over cores of min(1/local_max, 1e5), so the
                # post-collective path needs no reciprocal before s127
                mx2 = smal.tile([128, 1], f32)
                nc.vector.tensor_reduce(out=mx2[:], in_=mxbuf[:],
                                        axis=mybir.AxisListType.X, op=Alu.max)
                mxr = smal.tile([128, 1], f32)
                nc.gpsimd.partition_all_reduce(
                    mxr[:], mx2[:], 128, bass_isa.ReduceOp.max)
                mxs = smal.tile([128, 1], f32)
                nc.scalar.activation(out=mxs[:], in_=mxr[:], func=ACTF.Sqrt)
                mxi = smal.tile([128, 1], f32)
                nc.vector.reciprocal(mxi[:], mxs[:])
                nc.vector.tensor_scalar_min(mxi[:], mxi[:], 1e5)
                nc.gpsimd.dma_start(out=ccin[:], in_=mxi[0:1, 0:1])
                if use_ag:
                    nc.gpsimd.collective_compute(
                        "AllGather", Alu.bypass,
                        replica_groups=[list(range(N_CORES))],
                        ins=[ccin.opt()], outs=[ccag.opt()],
                    )
                else:
                    nc.gpsimd.collective_compute(
                        "AllReduce", Alu.min,
                        replica_groups=[list(range(N_CORES))],
                        ins=[ccin.opt()], outs=[ccag.opt()],
                    )

                # ---- weights: marker-gated DMA + |w| sums on ACT ----
                # the 1-elem copies from a late chunk's mxbuf column keep
                # the 7MiB weight DMA off the phase-A wire
                _markers = os.environ.get("BITCONV_MARKERS", "1") == "1"
                wraws = []
                wg = wsmp.tile([128, KS * C], bf16)   # Abs garbage dst
                for m in range(NT):
                    wraw = wrawp.tile([128, KS * C], f32)
                    if _markers:
                        nc.vector.tensor_copy(out=wraw[0:1, 0:1],
                                              in_=mxbuf[0:1, 13:14])
                    nc.sync.dma_start(
                        out=wraw[:],
                        in_=w_ext[m * 128:(m + 1) * 128, :, :].rearrange(
                            "p k c -> p (k c)"))
                    wraws.append(wraw)
                    if os.environ.get("BITCONV_WABS", "1") == "1":
                        nc.scalar.activation(out=wg[:], in_=wraw[:],
                                             func=ACTF.Abs,
                                             accum_out=wsums[:, m:m + 1])
                    else:
                        nc.vector.tensor_reduce(
                            out=wsums[:, m:m + 1], in_=wraw[:],
                            axis=mybir.AxisListType.X, op=Alu.add,
                            apply_absolute_value=True)
                wtot = wsmp.tile([128, 1], f32)
                nc.vector.tensor_reduce(out=wtot[:], in_=wsums[:],
                                        axis=mybir.AxisListType.X, op=Alu.add)
                wtr = smal.tile([128, 1], f32)
                nc.gpsimd.partition_all_reduce(
                    wtr[:], wtot[:], 128, bass_isa.ReduceOp.add)
                nc.vector.tensor_scalar(out=ws128[:], in0=wtr[:],
                                        scalar1=1.0 / W_ELEMS, scalar2=1e-4,
                                        op0=Alu.mult, op1=Alu.max)
                nc.vector.reciprocal(winv[:], ws128[:])

                # ---- weight quantize: ACT round, DVE clip + sub->bf16 ----
                for m in range(NT):
                    nc.scalar.activation(out=wraws[m][:], in_=wraws[m][:],
                                         func=ACTF.Copy, scale=winv[:],
                                         bias=MAGIC)
                    nc.vector.tensor_scalar(out=wraws[m][:], in0=wraws[m][:],
                                            scalar1=MAGIC + 1.0,
                                            scalar2=MAGIC - 1.0,
                                            op0=Alu.min, op1=Alu.max)
                    nc.vector.tensor_scalar_sub(wqTs[m][:], wraws[m][:],
                                                MAGIC)

                # ---- dummy matmuls keep the PE HAM warm over the window ----
                if warm > 0:
                    nc.vector.tensor_copy(out=dumr[0:1, 0:1],
                                          in_=mxbuf[0:1, 15:16])
                    dps = psD.tile([128, 256], f32)
                    for _ in range(warm):
                        nc.tensor.matmul(dps[:], ones_h[:, 0:128], dumr[:],
                                         start=True, stop=True)

                # ---- post-collective scalar setup (sinv = 1/scale) ----
                # ALL on ACT/gpsimd: a CC-dependent DVE op would head-of-line
                # block ready weight/quantize work in the DVE queue (Tile's
                # static order assumes the collective is instant)
                sinv = smal.tile([128, 1], f32)
                if use_ag:
                    agt = smal.tile([1, N_CORES], f32)
                    nc.gpsimd.dma_start(out=agt[:],
                                        in_=ccag[:].rearrange("r o -> o r"))
                    scs = smal.tile([1, 1], f32)
                    nc.gpsimd.tensor_reduce(out=scs[:], in_=agt[:],
                                            axis=mybir.AxisListType.XYZWC,
                                            op=Alu.min)
                    nc.gpsimd.dma_start(out=ccsc[:], in_=scs[:])
                    nc.sync.dma_start(out=sinv[:],
                                      in_=ccsc[:].to_broadcast((128, 1)))
                else:
                    nc.sync.dma_start(out=sinv[:],
                                      in_=ccag[:].to_broadcast((128, 1)))
                nc.scalar.activation(out=s127[:], in_=sinv[:],
                                     func=ACTF.Copy, scale=QP)
                if apply_gamma:
                    nc.scalar.activation(out=gs4[:], in_=gamma4[:],
                                         func=ACTF.Copy, scale=s127[:])
                # osc = ws*scale/127; 1/sinv via the ACT rsqrt table on
                # sinv^2 (4e-5 systematic, well under budget); first needed
                # ~6us later at the first PSUM evac
                sv2 = smal.tile([128, 1], f32)
                nc.scalar.square(sv2[:], sinv[:])
                nc.scalar.activation(out=sc128[:], in_=sv2[:],
                                     func=ACTF.Abs_reciprocal_sqrt)
                nc.scalar.activation(out=osc[:], in_=ws128[:],
                                     func=ACTF.Copy, scale=sc128[:])
                nc.scalar.activation(out=osc[:], in_=osc[:],
                                     func=ACTF.Copy, scale=1.0 / QP)

            # ---------------- phase B: quantize + conv matmuls ---------------
            with (
                tc.tile_pool(name="xh", bufs=3) as xhp,
                tc.tile_pool(name="qm", bufs=3) as qmp,
                tc.tile_pool(name="qf", bufs=2) as qfp,
                tc.tile_pool(name="nb", bufs=6) as nbp,
                tc.tile_pool(name="ob", bufs=4) as obp,
                tc.tile_pool(name="psC", bufs=8, space="PSUM") as psC,
            ):
                for ti in range(NCH):
                    t0 = ti * CH
                    lo = max(t0 - PAD, 0)
                    hi = min(t0 + CH + PAD, T)
                    dst_lo = lo - (t0 - PAD)
                    dst_hi = dst_lo + (hi - lo)
                    xh = xhp.tile([128, NT, HB], f32)
                    if os.environ.get("BITCONV_MARKERS", "1") == "1":
                        # marker: keep phase-B x reloads off the phase-A wire
                        nc.vector.tensor_copy(out=xh[0:1, 0, 0:1],
                                              in_=mxbuf[0:1, 15:16])
                    if dst_lo > 0:
                        nc.vector.memset(xh[:, :, 0:dst_lo], 0.0)
                    if dst_hi < HB:
                        nc.vector.memset(xh[:, :, dst_hi:HB], 0.0)
                    nc.sync.dma_start(
                        out=xh[:, :, dst_lo:dst_hi],
                        in_=x_ext[:, lo:hi].rearrange("(j p) t -> p j t",
                                                      p=128))
                    # x*rms is scale-free: Tile runs the first chunks'
                    # multiplies inside the collective window
                    qm = qmp.tile([128, NT, HB], f32)
                    rsl = rms_res[:, t0 + MPAD - PAD:t0 + MPAD - PAD + HB]
                    nc.vector.tensor_mul(
                        qm[:], xh[:],
                        rsl[:, None, :].broadcast_to([128, NT, HB]))
                    qf = qfp.tile([128, NT, HB], f32)
                    if apply_gamma:
                        for j in range(NT):
                            nc.scalar.activation(out=qf[:, j, :],
                                                 in_=qm[:, j, :],
                                                 func=ACTF.Copy,
                                                 scale=gs4[:, j:j + 1],
                                                 bias=MAGIC)
                    else:
                        nc.scalar.activation(out=qf[:], in_=qm[:],
                                             func=ACTF.Copy,
                                             scale=s127[:], bias=MAGIC)
                    # two copies: even-k taps read nb, odd-k taps read nb1
                    # (shifted 1 elem) so every matmul rhs slice is 4-byte
                    # aligned (odd bf16 offsets fault the PE).
                    nb = nbp.tile([128, NT, HB], bf16)
                    nc.vector.tensor_scalar_sub(nb[:], qf[:], MAGIC)
                    nb1 = nbp.tile([128, NT, HB], bf16)
                    nc.vector.tensor_copy(out=nb1[:, :, 0:HB - 1],
                                          in_=nb[:, :, 1:HB])
                    for m in range(NT):
                        pc = psC.tile([128, CH], f32)
                        idx = 0
                        for par in (0, 1):
                            for j in range(NT):
                                for k in range(par, KS, 2):
                                    w_sl = wqTs[j][:, k * C + m * 128:
                                                   k * C + m * 128 + 128]
                                    if par == 0:
                                        rhs = nb[:, j, k:k + CH]
                                    else:
                                        rhs = nb1[:, j, k - 1:k - 1 + CH]
                                    nc.tensor.matmul(
                                        pc[:], w_sl, rhs,
                                        start=(idx == 0),
                                        stop=(idx == NT * KS - 1))
                                    idx += 1
                        ob = obp.tile([128, CH], f32)
                        nc.scalar.activation(out=ob[:], in_=pc[:],
                                             func=ACTF.Copy, scale=osc[:])
                        nc.sync.dma_start(
                            out=out_ext[m * 128:(m + 1) * 128,
                                        t0:t0 + CH],
                            in_=ob[:])

    nc.finalize()
    return nc


_NC_CACHE = {}


def _get_nc(apply_gamma: bool):
    key = (apply_gamma, os.environ.get("BITCONV_AG", "0"),
           os.environ.get("BITCONV_WARM", "430"),
           os.environ.get("BITCONV_MARKERS", "1"),
           os.environ.get("BITCONV_TTR", "0"),
           os.environ.get("BITCONV_WABS", "1"))
    if key not in _NC_CACHE:
        _NC_CACHE[key] = _build(apply_gamma)
    return _NC_CACHE[key]


def _run(x, weight, norm_weight, trace=False, tmpdir=None):
    x = np.ascontiguousarray(x, dtype=np.float32)
    weight = np.ascontiguousarray(weight, dtype=np.float32)
    norm_weight = np.ascontiguousarray(norm_weight, dtype=np.float32)
    assert x.shape == (N_CORES, C, T), x.shape
    assert weight.shape == (C, C, KS), weight.shape
    assert norm_weight.shape == (C,), norm_weight.shape
    # device wants lhsT layout [cin, k, cout] (pure layout permutation)
    weight = np.ascontiguousarray(weight.transpose(1, 2, 0))

    apply_gamma = not bool(np.all(norm_weight == np.float32(1.0)))
    nc = _get_nc(apply_gamma)
    in_maps = [
        {"x": x[i], "w": weight, "nw": norm_weight} for i in range(N_CORES)
    ]
    res = run_bass_kernel_spmd(nc, in_maps, list(range(N_CORES)),
                               trace=trace, tmpdir=tmpdir)
    out = np.stack([res.results[i]["out"] for i in range(N_CORES)], axis=0)
    return out, res.exec_time_ns


def kernel(x, weight, norm_weight):
    out, _ = _run(x, weight, norm_weight)
    return out


# revision 32
# speedup vs baseline: 1.0253x; 1.0253x over previous
"""BitConv1d Trainium2 kernel (8 NeuronCores, data-parallel over batch).

Reference semantics (per batch b):
    x_n   = rmsnorm_over_C(x) * gamma
    scale = max(|x_n|) over the WHOLE tensor (global -> AllGather max)
    n     = round(x_n / scale * 127)                          (ints, |n|<=127)
    w_s   = max(mean(|w|), 1e-4)
    w_q   = round(clip(w / w_s, -1, 1))                       (ternary)
    out   = conv1d(n, w_q, pad=3) * (scale/127) * w_s

n is an integer |n|<=127 (exact in bf16) and w_q is ternary (exact in
bf16), so the conv is EXACT integer arithmetic on the PE in bf16 with
fp32 PSUM accumulation.  Rounding uses the fp32 magic-number trick
(+1.5*2^23, RNE) which matches jnp.round.

v3 structure (evidence from perfetto traces of v1/v2):
  * Only rms (1/sqrt(mean x^2 + eps), [128, T] f32, 32KiB/part) stays
    resident in SBUF.  Phase A computes rms + the global abs-max with a
    SINGLE DVE scan per chunk: tensor_tensor_reduce(sq * rms^2,
    accum=max) gives max(x_n^2) while writing only a garbage tile.
    Phase B re-reads x from DRAM (the extra 16MiB hides under 460us of
    matmuls) and computes x*rms on the DVE there.
  * Tile schedules by operand readiness, not emission order, so the
    weight DMAs + phase-B x DMAs carry dependency markers (1-element
    copies from late-phase-A tiles) to keep them off the phase-A DMA
    wire; the weight-quant chain (ACT |w|-accum, ACT magic-round, DVE
    clip/sub) then lands inside the ~45us collective window.
  * The global-max tree uses gpsimd.partition_all_reduce (lib preloaded
    at t=0 via a dummy call); the post-collective scalar broadcast is a
    stride-0 DMA (dram scalar .to_broadcast).
  * Dummy matmuls (dep-chained to the end of phase A) keep the PE HAM
    clock-gate warm through the collective window.
  * Phase B runs chunk-major with 8 PSUM banks double-buffered; the
    scale-free x*rms multiply of the first chunks runs during the
    collective window as soon as their x slices land.
"""

import os
import sys
import types

import numpy as np


def _install_ntff_shim():
    """Make bass_utils' trace path work in containers lacking antenv.axon_hooks."""
    try:
        import antenv.axon_hooks  # noqa: F401
        return
    except ImportError:
        pass
    try:
        from trn_agent_boot.trn_boot import _ntff_profile_via_ctypes

        mod = types.ModuleType("antenv.axon_hooks")
        hook = _ntff_profile_via_ctypes("/opt/axon/libaxon_pjrt.so")
        mod.get_axon_ntff_profile_hook = lambda: hook
        mod.set_axon_ntff_profile_hook = lambda h: None
        sys.modules["antenv.axon_hooks"] = mod
        import antenv

        antenv.axon_hooks = mod
    except Exception:
        pass


_install_ntff_shim()

import concourse.bacc as bacc
import concourse.tile as tile
from concourse import bass_isa, mybir
from concourse.bass_utils import run_bass_kernel_spmd

f32 = mybir.dt.float32
fp16 = mybir.dt.float16
bf16 = mybir.dt.bfloat16

N_CORES = 8
C = 512          # in/out channels
T = 8192         # sequence length
KS = 7           # kernel taps
PAD = 3
NT = 4           # channel tiles of 128
CH = 512         # T-chunk width
NCH = T // CH    # 16
MPAD = 4         # rms margin (any even >= PAD)
TP = T + 2 * MPAD
HB = CH + 2 * PAD  # 518
EPS = 1e-6
QP = 127.0
MAGIC = 12582912.0        # 1.5 * 2**23 : fp32 round-to-nearest-int magic
W_ELEMS = C * C * KS      # 1835008


def _build(apply_gamma: bool):
    Alu = mybir.AluOpType
    ACTF = mybir.ActivationFunctionType
    warm = int(os.environ.get("BITCONV_WARM", "430"))
    use_ag = os.environ.get("BITCONV_AG", "0") == "1"

    nc = bacc.Bacc("TRN2", target_bir_lowering=False, debug=False,
                   num_devices=N_CORES)

    x_ext = nc.dram_tensor("x", [C, T], f32, kind="ExternalInput")
    # host supplies weight transposed to [cin, k, cout] so quantized lhsT
    # tiles are contiguous slices (no on-chip transposes needed)
    w_ext = nc.dram_tensor("w", [C, KS, C], f32, kind="ExternalInput")
    nw_ext = nc.dram_tensor("nw", [C], f32, kind="ExternalInput")
    out_ext = nc.dram_tensor("out", [C, T], f32, kind="ExternalOutput")

    with tile.TileContext(nc) as tc:
        with (
            tc.tile_pool(name="consts", bufs=1) as consts,
            tc.tile_pool(name="rmsres", bufs=1) as rmsres,
            tc.tile_pool(name="wqt", bufs=1) as wqtp,
            tc.tile_pool(name="dram", bufs=1, space="DRAM") as dram,
        ):
            ones_h = consts.tile([128, 256], fp16)
            nc.vector.memset(ones_h[:], 1.0)
            eps_t = consts.tile([128, 1], f32)
            nc.vector.memset(eps_t[:], EPS)
            gamma4 = consts.tile([128, NT], f32)
            if apply_gamma:
                nc.sync.dma_start(
                    out=gamma4[:],
                    in_=nw_ext[:].rearrange("(j p) -> p j", p=128))
            mxbuf = consts.tile([128, NCH], f32)     # max(x_n^2) per chunk
            wsums = consts.tile([128, NT], f32)
            sc128 = consts.tile([128, 1], f32)       # global act scale
            s127 = consts.tile([128, 1], f32)        # 127/scale
            gs4 = consts.tile([128, NT], f32)        # gamma * 127/scale
            ws128 = consts.tile([128, 1], f32)       # weight scale
            winv = consts.tile([128, 1], f32)
            osc = consts.tile([128, 1], f32)         # w_s*scale/127
            scr1 = consts.tile([128, 1], f32)        # gpsimd lib preload dst
            dumr = consts.tile([128, 256], fp16)     # dummy-matmul rhs
            nc.vector.memset(dumr[:], 1.0)

            # resident rms, f32, zero margins so the phase-B halo mul
            # produces exact zeros outside [0, T)
            rms_res = rmsres.tile([128, TP], f32)
            nc.vector.memset(rms_res[:, 0:MPAD], 0.0)
            nc.vector.memset(rms_res[:, MPAD + T:TP], 0.0)

            # ternary weights, bf16, lhsT layout: tile j holds
            # [128 cin, (k, cout)] so slice (k, m) is contiguous
            wqTs = [wqtp.tile([128, KS * C], bf16, name=f"wqT{j}")
                    for j in range(NT)]

            ccin = dram.tile([1, 1], f32)
            if use_ag:
                ccag = dram.tile([N_CORES, 1], f32, addr_space="Shared")
                ccsc = dram.tile([1, 1], f32)
            else:
                ccag = dram.tile([1, 1], f32)

            with (
                tc.tile_pool(name="xin", bufs=3) as xinp,
                tc.tile_pool(name="sq", bufs=2) as sqp,
                tc.tile_pool(name="rms2", bufs=2) as rms2p,
                tc.tile_pool(name="g2", bufs=1) as g2p,
                tc.tile_pool(name="wraw", bufs=4) as wrawp,
                tc.tile_pool(name="wsm", bufs=2) as wsmp,
                tc.tile_pool(name="smal", bufs=4) as smal,
                tc.tile_pool(name="psA", bufs=3, space="PSUM") as psA,
                tc.tile_pool(name="psD", bufs=1, space="PSUM") as psD,
            ):
                # preload the gpsimd custom-op library at t~0 so the
                # partition_all_reduce calls later don't pay the
                # LOAD_LIB + DRAIN (~6us) inside the critical window
                nc.gpsimd.partition_all_reduce(
                    scr1[:], eps_t[:], 128, bass_isa.ReduceOp.max)

                g2 = g2p.tile([128, NT, CH], f32)    # scan garbage dst

                # ---- phase A: rms into resident f32, chunk max(x_n^2) ----
                for ti in range(NCH):
                    t0 = ti * CH
                    xt = xinp.tile([128, NT, CH], f32)
                    nc.sync.dma_start(
                        out=xt[:],
                        in_=x_ext[:, t0:t0 + CH].rearrange(
                            "(j p) t -> p j t", p=128))
                    sq = sqp.tile([128, NT, CH], fp16)
                    nc.scalar.square(sq[:], xt[:])
                    ps = psA.tile([128, CH], f32)
                    for j in range(NT):
                        # accumulate sum_c x^2 on the PE; all-ones lhsT also
                        # broadcasts the result to every partition
                        nc.tensor.matmul(ps[:], ones_h[:, 0:128], sq[:, j, :],
                                         start=(j == 0), stop=(j == NT - 1))
                    rsl = rms_res[:, MPAD + t0:MPAD + t0 + CH]
                    # table rsqrt (max rel err ~4e-5 measured)
                    nc.scalar.activation(out=rsl, in_=ps[:],
                                         func=ACTF.Abs_reciprocal_sqrt,
                                         bias=eps_t[:], scale=1.0 / C)
                    rms2 = rms2p.tile([128, CH], f32)
                    nc.scalar.square(rms2[:], rsl)
                    if apply_gamma:
                        # chunk max must cover gamma^2 * x^2 * rms^2; fold
                        # gamma into phase-B scale, but the max needs it
                        # per-channel -> use gamma'd squares via ttr twice
                        gb2 = smal.tile([128, NT], f32)
                        nc.vector.tensor_mul(gb2[:], gamma4[:], gamma4[:])
                        sqg = sqp.tile([128, NT, CH], fp16)
                        nc.vector.tensor_mul(
                            sqg[:], sq[:],
                            gb2[:, :, None].broadcast_to([128, NT, CH]))
                        nc.vector.tensor_tensor_reduce(
                            out=g2[:], in0=sqg[:],
                            in1=rms2[:, None, :].broadcast_to([128, NT, CH]),
                            scale=1.0, scalar=0.0,
                            op0=Alu.mult, op1=Alu.max,
                            accum_out=mxbuf[:, ti:ti + 1])
                    else:
                        if os.environ.get("BITCONV_TTR", "0") == "1":
                            # ONE DVE scan: garbage out, accum carries
                            # max(x^2 * rms^2) = max(x_n^2) for the chunk
                            nc.vector.tensor_tensor_reduce(
                                out=g2[:], in0=sq[:],
                                in1=rms2[:, None, :].broadcast_to(
                                    [128, NT, CH]),
                                scale=1.0, scalar=0.0,
                                op0=Alu.mult, op1=Alu.max,
                                accum_out=mxbuf[:, ti:ti + 1])
                        else:
                            nc.vector.tensor_mul(
                                g2[:], sq[:],
                                rms2[:, None, :].broadcast_to([128, NT, CH]))
                            # flat view: one AP row, no per-row overhead
                            nc.vector.tensor_reduce(
                                out=mxbuf[:, ti:ti + 1],
                                in_=g2[:].rearrange("p a b -> p (a b)"),
                                axis=mybir.AxisListType.X, op=Alu.max)

                # ---- local max tree + collective (gpsimd only) ----
                # the wire carries the CLAMPED INVERSE scale: global
                # 1/scale = min over cores of min(1/local_max, 1e5), so the
                # post-collective path needs no reciprocal before s127
                mx2 = smal.tile([128, 1], f32)
                nc.vector.tensor_reduce(out=mx2[:], in_=mxbuf[:],
                                        axis=mybir.AxisListType.X, op=Alu.max)
                mxr = smal.tile([128, 1], f32)
                nc.gpsimd.partition_all_reduce(
                    mxr[:], mx2[:], 128, bass_isa.ReduceOp.max)
                mxs = smal.tile([128, 1], f32)
                nc.scalar.activation(out=mxs[:], in_=mxr[:], func=ACTF.Sqrt)
                mxi = smal.tile([128, 1], f32)
                nc.vector.reciprocal(mxi[:], mxs[:])
                nc.vector.tensor_scalar_min(mxi[:], mxi[:], 1e5)
                nc.gpsimd.dma_start(out=ccin[:], in_=mxi[0:1, 0:1])
                if use_ag:
                    nc.gpsimd.collective_compute(
                        "AllGather", Alu.bypass,
                        replica_groups=[list(range(N_CORES))],
                        ins=[ccin.opt()], outs=[ccag.opt()],
                    )
                else:
                    nc.gpsimd.collective_compute(
                        "AllReduce", Alu.min,
                        replica_groups=[list(range(N_CORES))],
                        ins=[ccin.opt()], outs=[ccag.opt()],
                    )

                # ---- weights: marker-gated DMA + |w| sums on ACT ----
                # the 1-elem copies from a late chunk's mxbuf column keep
                # the 7MiB weight DMA off the phase-A wire
                _markers = os.environ.get("BITCONV_MARKERS", "1") == "1"
                wraws = []
                wg = wsmp.tile([128, KS * C], bf16)   # Abs garbage dst
                for m in range(NT):
                    wraw = wrawp.tile([128, KS * C], f32)
                    if _markers:
                        nc.vector.tensor_copy(out=wraw[0:1, 0:1],
                                              in_=mxbuf[0:1, 13:14])
                    nc.sync.dma_start(
                        out=wraw[:],
                        in_=w_ext[m * 128:(m + 1) * 128, :, :].rearrange(
                            "p k c -> p (k c)"))
                    wraws.append(wraw)
                    if os.environ.get("BITCONV_WABS", "1") == "1":
                        nc.scalar.activation(out=wg[:], in_=wraw[:],
                                             func=ACTF.Abs,
                                             accum_out=wsums[:, m:m + 1])
                    else:
                        nc.vector.tensor_reduce(
                            out=wsums[:, m:m + 1], in_=wraw[:],
                            axis=mybir.AxisListType.X, op=Alu.add,
                            apply_absolute_value=True)
                wtot = wsmp.tile([128, 1], f32)
                nc.vector.tensor_reduce(out=wtot[:], in_=wsums[:],
                                        axis=mybir.AxisListType.X, op=Alu.add)
                wtr = smal.tile([128, 1], f32)
                nc.gpsimd.partition_all_reduce(
                    wtr[:], wtot[:], 128, bass_isa.ReduceOp.add)
                nc.vector.tensor_scalar(out=ws128[:], in0=wtr[:],
                                        scalar1=1.0 / W_ELEMS, scalar2=1e-4,
                                        op0=Alu.mult, op1=Alu.max)
                nc.vector.reciprocal(winv[:], ws128[:])

                # ---- weight quantize: ACT round, DVE clip + sub->bf16 ----
                for m in range(NT):
                    nc.scalar.activation(out=wraws[m][:], in_=wraws[m][:],
                                         func=ACTF.Copy, scale=winv[:],
                                         bias=MAGIC)
                    nc.vector.tensor_scalar(out=wraws[m][:], in0=wraws[m][:],
                                            scalar1=MAGIC + 1.0,
                                            scalar2=MAGIC - 1.0,
                                            op0=Alu.min, op1=Alu.max)
                    nc.vector.tensor_scalar_sub(wqTs[m][:], wraws[m][:],
                                                MAGIC)

                # ---- dummy matmuls keep the PE HAM warm over the window ----
                if warm > 0:
                    nc.vector.tensor_copy(out=dumr[0:1, 0:1],
                                          in_=mxbuf[0:1, 15:16])
                    dps = psD.tile([128, 256], f32)
                    for _ in range(warm):
                        nc.tensor.matmul(dps[:], ones_h[:, 0:128], dumr[:],
                                         start=True, stop=True)

                # ---- post-collective scalar setup (sinv = 1/scale) ----
                # ALL on ACT/gpsimd: a CC-dependent DVE op would head-of-line
                # block ready weight/quantize work in the DVE queue (Tile's
                # static order assumes the collective is instant)
                sinv = smal.tile([128, 1], f32)
                if use_ag:
                    agt = smal.tile([1, N_CORES], f32)
                    nc.gpsimd.dma_start(out=agt[:],
                                        in_=ccag[:].rearrange("r o -> o r"))
                    scs = smal.tile([1, 1], f32)
                    nc.gpsimd.tensor_reduce(out=scs[:], in_=agt[:],
                                            axis=mybir.AxisListType.XYZWC,
                                            op=Alu.min)
                    nc.gpsimd.dma_start(out=ccsc[:], in_=scs[:])
                    nc.sync.dma_start(out=sinv[:],
                                      in_=ccsc[:].to_broadcast((128, 1)))
                else:
                    nc.sync.dma_start(out=sinv[:],
                                      in_=ccag[:].to_broadcast((128, 1)))
                nc.scalar.activation(out=s127[:], in_=sinv[:],
                                     func=ACTF.Copy, scale=QP)
                if apply_gamma:
                    nc.scalar.activation(out=gs4[:], in_=gamma4[:],
                                         func=ACTF.Copy, scale=s127[:])
                # osc = ws*scale/127; 1/sinv via the ACT rsqrt table on
                # sinv^2 (4e-5 systematic, well under budget); first needed
                # ~6us later at the first PSUM evac
                sv2 = smal.tile([128, 1], f32)
                nc.scalar.square(sv2[:], sinv[:])
                nc.scalar.activation(out=sc128[:], in_=sv2[:],
                                     func=ACTF.Abs_reciprocal_sqrt)
                nc.scalar.activation(out=osc[:], in_=ws128[:],
                                     func=ACTF.Copy, scale=sc128[:])
                nc.scalar.activation(out=osc[:], in_=osc[:],
                                     func=ACTF.Copy, scale=1.0 / QP)

            # ---------------- phase B: quantize + conv matmuls ---------------
            with (
                tc.tile_pool(name="xh", bufs=3) as xhp,
                tc.tile_pool(name="qm", bufs=3) as qmp,
                tc.tile_pool(name="qf", bufs=2) as qfp,
                tc.tile_pool(name="nb", bufs=6) as nbp,
                tc.tile_pool(name="ob", bufs=4) as obp,
                tc.tile_pool(name="psC", bufs=8, space="PSUM") as psC,
            ):
                def quantize_chunk(ti):
                    t0 = ti * CH
                    lo = max(t0 - PAD, 0)
                    hi = min(t0 + CH + PAD, T)
                    dst_lo = lo - (t0 - PAD)
                    dst_hi = dst_lo + (hi - lo)
                    xh = xhp.tile([128, NT, HB], f32)
                    if os.environ.get("BITCONV_MARKERS", "1") == "1":
                        # marker: keep phase-B x reloads off the phase-A wire
                        nc.vector.tensor_copy(out=xh[0:1, 0, 0:1],
                                              in_=mxbuf[0:1, 15:16])
                    if dst_lo > 0:
                        nc.vector.memset(xh[:, :, 0:dst_lo], 0.0)
                    if dst_hi < HB:
                        nc.vector.memset(xh[:, :, dst_hi:HB], 0.0)
                    nc.sync.dma_start(
                        out=xh[:, :, dst_lo:dst_hi],
                        in_=x_ext[:, lo:hi].rearrange("(j p) t -> p j t",
                                                      p=128))
                    # x*rms is scale-free: Tile runs the first chunks'
                    # multiplies inside the collective window
                    qm = qmp.tile([128, NT, HB], f32)
                    rsl = rms_res[:, t0 + MPAD - PAD:t0 + MPAD - PAD + HB]
                    nc.vector.tensor_mul(
                        qm[:], xh[:],
                        rsl[:, None, :].broadcast_to([128, NT, HB]))
                    qf = qfp.tile([128, NT, HB], f32)
                    if apply_gamma:
                        for j in range(NT):
                            nc.scalar.activation(out=qf[:, j, :],
                                                 in_=qm[:, j, :],
                                                 func=ACTF.Copy,
                                                 scale=gs4[:, j:j + 1],
                                                 bias=MAGIC)
                    else:
                        nc.scalar.activation(out=qf[:], in_=qm[:],
                                             func=ACTF.Copy,
                                             scale=s127[:], bias=MAGIC)
                    # two copies: even-k taps read nb, odd-k taps read nb1
                    # (shifted 1 elem) so every matmul rhs slice is 4-byte
                    # aligned (odd bf16 offsets fault the PE).
                    nb = nbp.tile([128, NT, HB], bf16)
                    nc.vector.tensor_scalar_sub(nb[:], qf[:], MAGIC)
                    nb1 = nbp.tile([128, NT, HB], bf16)
                    nc.vector.tensor_copy(out=nb1[:, :, 0:HB - 1],
                                          in_=nb[:, :, 1:HB])
                    return nb, nb1

                nbs = quantize_chunk(0)
                for ti in range(NCH):
                    t0 = ti * CH
                    nb, nb1 = nbs
                    evacs = []
                    for m in range(NT):
                        pc = psC.tile([128, CH], f32)
                        idx = 0
                        for par in (0, 1):
                            for j in range(NT):
                                for k in range(par, KS, 2):
                                    w_sl = wqTs[j][:, k * C + m * 128:
                                                   k * C + m * 128 + 128]
                                    if par == 0:
                                        rhs = nb[:, j, k:k + CH]
                                    else:
                                        rhs = nb1[:, j, k - 1:k - 1 + CH]
                                    nc.tensor.matmul(
                                        pc[:], w_sl, rhs,
                                        start=(idx == 0),
                                        stop=(idx == NT * KS - 1))
                                    idx += 1
                        evacs.append((m, pc))
                    # quantize the NEXT chunk before this chunk's PSUM
                    # evacuations so the ACT queue can't park it behind them
                    if ti + 1 < NCH:
                        nbs = quantize_chunk(ti + 1)
                    for m, pc in evacs:
                        ob = obp.tile([128, CH], f32)
                        nc.scalar.activation(out=ob[:], in_=pc[:],
                                             func=ACTF.Copy, scale=osc[:])
                        nc.sync.dma_start(
                            out=out_ext[m * 128:(m + 1) * 128,
                                        t0:t0 + CH],
                            in_=ob[:])

    nc.finalize()
    return nc


_NC_CACHE = {}


def _get_nc(apply_gamma: bool):
    key = (apply_gamma, os.environ.get("BITCONV_AG", "0"),
           os.environ.get("BITCONV_WARM", "430"),
           os.environ.get("BITCONV_MARKERS", "1"),
           os.environ.get("BITCONV_TTR", "0"),
           os.environ.get("BITCONV_WABS", "1"))
    if key not in _NC_CACHE:
        _NC_CACHE[key] = _build(apply_gamma)
    return _NC_CACHE[key]


def _run(x, weight, norm_weight, trace=False, tmpdir=None):
    x = np.ascontiguousarray(x, dtype=np.float32)
    weight = np.ascontiguousarray(weight, dtype=np.float32)
    norm_weight = np.ascontiguousarray(norm_weight, dtype=np.float32)
    assert x.shape == (N_CORES, C, T), x.shape
    assert weight.shape == (C, C, KS), weight.shape
    assert norm_weight.shape == (C,), norm_weight.shape
    # device wants lhsT layout [cin, k, cout] (pure layout permutation)
    weight = np.ascontiguousarray(weight.transpose(1, 2, 0))

    apply_gamma = not bool(np.all(norm_weight == np.float32(1.0)))
    nc = _get_nc(apply_gamma)
    in_maps = [
        {"x": x[i], "w": weight, "nw": norm_weight} for i in range(N_CORES)
    ]
    res = run_bass_kernel_spmd(nc, in_maps, list(range(N_CORES)),
                               trace=trace, tmpdir=tmpdir)
    out = np.stack([res.results[i]["out"] for i in range(N_CORES)], axis=0)
    return out, res.exec_time_ns


def kernel(x, weight, norm_weight):
    out, _ = _run(x, weight, norm_weight)
    return out
